# revision 85
# baseline (speedup 1.0000x reference)
"""MultiHeadLinearAttention (Linformer-style) on 8 trn2 NeuronCores.

Strategy (head-parallel attention + per-(batch,head) AllToAll +
token-parallel output projection with post-exchange normalization):
  - 16 heads -> 8 cores: 2 heads (one d_model slice of 128) per core.
  - Phase 1, per core: Kp [128(d2), 256k] = K_slice^T @ We (+be) and
    Vp [256k, 128(d2)] = Wf^T @ V_slice (+bf), contiguous streamed loads.
  - Phase 2, per (b, h, 512-token block nh):
      s^T [256k, 512n] = Kp_h^T @ Q_h^T  (zero-padded to uniform
          (128,128) PE tiles: mixed tile geometries stall the PE stream)
      E^T = exp(s^T / 8)                 (Act engine, per-kc tiles)
      at  = [Vp_h | one | 0pad]^T @ E^T: rows 0:64 = unnormalized
          attention numerator^T, row 64 = softmax denominator (free
          via the ones column).
      The 65-row block ships UNNORMALIZED through the AllToAll.
  - Per (b, h) AllToAll (8 half-size collectives; heads are the outer
    phase-2 loop so each batch's first collective triggers at its
    half-way point -> the serial collective stream starts earlier and
    its tail transfer is half-sized).
  - Phase 3, per b (token-sharded): batch all 16 denominator rows,
    one bf16 reciprocal, bounce through DRAM and replicate each row to
    64 partitions with stride-0 broadcast DMAs, normalize g-tiles on
    Pool+DVE, then out[n_shard] = gn^T @ Wo (+bo).

Scheduling notes: every phase-3 instruction is order-pinned behind
phase 2 of batch b+2 on its own engine/queue (add_dep_helper) — the
tile scheduler's collective-latency estimate is optimistic, and an
early-placed instruction waiting on a slow AllToAll head-of-line
blocks its whole in-order engine stream.  A tiny warmup AllToAll
absorbs the ~12us first-collective rendezvous during phase 1.  DMAs
are spread across the two hardware DGE queues (scalar: K/V loads +
Wo + output stores; sync: everything else) so streams run in
parallel.  All matmuls bf16 with fp32 PSUM accumulation.
"""

import numpy as np
import ml_dtypes

import concourse.bass as bass
import concourse.mybir as mybir
from concourse.tile import TileContext
from concourse.bass_utils import run_bass_kernel_spmd
from concourse.tile_rust import add_dep_helper

B, N, D, H, LK = 4, 4096, 1024, 16, 256
DK = D // H          # 64
NC = 8               # cores
NSH = N // NC        # 512 tokens per (core, nh-block)
P = 128
NCH = 32             # 128-row chunks of N
NPH = 8              # 512-col chunks of N

F32 = mybir.dt.float32
BF16 = mybir.dt.bfloat16
NP_BF16 = ml_dtypes.bfloat16

_BUILD_CACHE = {}

_ws_ctr = [0]


def _split_multi_waits(nc, lim=1):
    """Walrus codegen on this stack rejects instructions whose on_wait list
    exceeds the per-format wait-slot count ("Too many sync wait commands").
    Engines execute in order, so excess waits move onto preceding NOPs on
    the same engine with identical semantics."""
    for f in nc.m.functions:
        for blk in f.blocks:
            insts = blk.instructions
            if not any(
                ins.sync_info is not None and len(ins.sync_info.on_wait or []) > lim
                for ins in insts
            ):
                continue
            out = []
            for ins in insts:
                si = ins.sync_info
                waits = list(si.on_wait) if si is not None and si.on_wait else []
                if len(waits) > lim and ins.engine is not None:
                    keep = waits[-lim:]
                    rest = waits[:-lim]
                    while rest:
                        chunk, rest = rest[:lim], rest[lim:]
                        _ws_ctr[0] += 1
                        nop = mybir.InstNoOp(
                            name=f"I-waitsplit-{_ws_ctr[0]}", ins=[], outs=[]
                        )
                        nop.engine = ins.engine
                        nop.sync_info = mybir.SyncInfo(on_wait=chunk, on_update=[])
                        out.append(nop)
                    ins.sync_info = mybir.SyncInfo(
                        on_wait=keep, on_update=list(si.on_update or [])
                    )
                out.append(ins)
            blk.instructions = out
    return nc


def _build(use_be, use_bf, use_bo):
    nc = bass.Bass(num_devices=NC)

    Ks_p = nc.declare_dram_parameter("Ks", [N, B, P], BF16, isOutput=False)
    Vs_p = nc.declare_dram_parameter("Vs", [N, B, P], BF16, isOutput=False)
    QT_p = nc.declare_dram_parameter("QTs", [B, P, N], BF16, isOutput=False)
    We_p = nc.declare_dram_parameter("WeS", [P, NCH, LK], BF16, isOutput=False)
    Wf_p = nc.declare_dram_parameter("WfS", [P, NCH, LK], BF16, isOutput=False)
    Wo_p = nc.declare_dram_parameter("WoS", [P, D // P, D], BF16, isOutput=False)
    if use_be:
        be_p = nc.declare_dram_parameter("beB", [P, LK], F32, isOutput=False)
    if use_bf:
        bf_p = nc.declare_dram_parameter("bfB", [P, 2], F32, isOutput=False)
    if use_bo:
        bo_p = nc.declare_dram_parameter("boB", [P, D], F32, isOutput=False)
    out_p = nc.declare_dram_parameter("out", [B, NSH, D], F32, isOutput=True)

    rg = [list(range(NC))]

    with TileContext(nc) as tc:
        with (
            tc.tile_pool(name="wpool", bufs=1) as wpool,
            tc.tile_pool(name="state", bufs=1) as state,
            tc.tile_pool(name="dram", bufs=1, space="DRAM") as dram,
        ):
            # ---- persistent tiles
            We_sb = wpool.tile([P, NCH, LK], BF16)
            Wf_sb = wpool.tile([P, NCH, LK], BF16)
            Wo_sb = wpool.tile([P, D // P, D], BF16)
            # kp_pad[d(128, other head zeroed), b, h, kc, k128]: zero-padded
            # so every phase-2 matmul is a uniform (128,128) PE tile — mixed
            # tile geometries measurably slow the throttled PE stream
            kp_pad = state.tile([P, B, 2, 2, P], BF16)
            nc.gpsimd.memset(kp_pad[:], 0.0)
            # vp_aug[k128, kc, b, h, (d64 | one | zeros)]: col 64 of ones
            # makes the at-matmul emit the softmax denominator as row 64 for
            # free; cols 65.. are zero padding for uniform tile geometry
            vp_aug = state.tile([P, 2, B, 2, P], BF16)
            nc.gpsimd.memset(vp_aug[:], 0.0)
            nc.gpsimd.memset(vp_aug[:, :, :, :, DK : DK + 1], 1.0)


            # We/Wf chunks interleave with K/V loads on the sync queue so
            # the first phase-1 matmul can start after ~3 transfers.
            if use_be:
                be_sb = wpool.tile([P, LK], F32)
                nc.scalar.dma_start(be_sb[:], be_p[:])
            if use_bf:
                bf_sb = wpool.tile([P, 2], F32)
                nc.scalar.dma_start(bf_sb[:], bf_p[:])

            # tiny warmup AllToAll: absorbs the ~12us first-collective
            # rendezvous latency during phase 1 instead of phase 2
            warm_in = dram.tile([NC, 1, 16], BF16, name="warm_in")
            warm_out = dram.tile([NC, 1, 16], BF16, name="warm_out")
            nc.gpsimd.collective_compute(
                "AllToAll",
                mybir.AluOpType.bypass,
                replica_groups=rg,
                ins=[warm_in[:]],
                outs=[warm_out[:]],
            )

            # ---- per-(batch, head) A2A buffers: 65 rows per destination =
            # 64 unnormalized numerator rows + 1 denominator row.
            # Normalization happens after the exchange on the token-sharded
            # side where denominators can be batched.  Processing heads as
            # the outer phase-2 loop lets each batch trigger its first
            # collective at the half-way point, so the serial collective
            # stream starts earlier and its last (tail-critical) transfer
            # is half-sized.
            a2a_in = [
                [
                    dram.tile([NC, DK + 1, NSH], BF16, name=f"a2a_in{b}_{h}")
                    for h in range(2)
                ]
                for b in range(B)
            ]
            a2a_out = [
                [
                    dram.tile([NC, DK + 1, NSH], BF16, name=f"a2a_out{b}_{h}")
                    for h in range(2)
                ]
                for b in range(B)
            ]
            # reciprocal denominators staged in DRAM: the partition-
            # broadcast DMA needs a DRAM source for stride-0 replication
            rden_d = [
                dram.tile([2 * (D // P), NSH], BF16, name=f"rden{b}")
                for b in range(B)
            ]


            # ================= phase 1: Kp / Vp =================
            with (
                tc.tile_pool(name="p1", bufs=1) as p1,
                tc.tile_pool(name="p1ps", bufs=1, space="PSUM") as p1ps,
            ):
                kp_ps = [
                    p1ps.tile([P, LK], F32, name=f"kp{b}", tag=f"kp{b}")
                    for b in range(B)
                ]
                vp_ps = [
                    p1ps.tile([P, B * P], F32, name=f"vp{kc}", tag=f"vp{kc}")
                    for kc in range(2)
                ]
                cs = slice(0, 4)
                nc.sync.dma_start(We_sb[:, cs, :], We_p[:, cs, :])
                nc.sync.dma_start(Wf_sb[:, cs, :], Wf_p[:, cs, :])
                for ic2 in range(NCH // 2):
                    if ic2 % 2 == 0 and ic2 // 2 + 1 < NCH // 4:
                        # prefetch the next 4-chunk slab of We/Wf one slab
                        # ahead of its first use
                        ch = ic2 // 2 + 1
                        cs = slice(ch * 4, (ch + 1) * 4)
                        nc.sync.dma_start(We_sb[:, cs, :], We_p[:, cs, :])
                        nc.sync.dma_start(Wf_sb[:, cs, :], Wf_p[:, cs, :])
                    # K loads ride the scalar queue (idle during phase 1) so
                    # K and V stream in parallel instead of serializing on
                    # one hardware DGE queue
                    K4 = p1.tile([P, 2, B, P], BF16, name="K4", tag="K4", bufs=3)
                    nc.scalar.dma_start(
                        K4[:],
                        Ks_p[ic2 * 2 * P : (ic2 + 1) * 2 * P, :, :].rearrange(
                            "(i p) b d -> p i b d", p=P
                        ),
                    )
                    V4 = p1.tile([P, 2, B, P], BF16, name="V4", tag="V4", bufs=3)
                    nc.scalar.dma_start(
                        V4[:],
                        Vs_p[ic2 * 2 * P : (ic2 + 1) * 2 * P, :, :].rearrange(
                            "(i p) b d -> p i b d", p=P
                        ),
                    )
                    for i in range(2):
                        ic = ic2 * 2 + i
                        for b in range(B):
                            nc.tensor.matmul(
                                kp_ps[b][:],
                                K4[:, i, b, :],
                                We_sb[:, ic, :],
                                start=(ic == 0),
                                stop=(ic == NCH - 1),
                            )
                        for kc in range(2):
                            nc.tensor.matmul(
                                vp_ps[kc][:],
                                Wf_sb[:, ic, kc * P : (kc + 1) * P],
                                V4[:, i, :, :],
                                start=(ic == 0),
                                stop=(ic == NCH - 1),
                            )

                # epilogue: stage Kp/Vp into bf16 SBUF (Act engine; DVE is
                # the phase-2 pacing engine and Act is idle here)
                for b in range(B):
                    for h in range(2):
                        hp = slice(h * DK, (h + 1) * DK)
                        for kc in range(2):
                            ks = slice(kc * P, (kc + 1) * P)
                            if use_be:
                                nc.vector.tensor_tensor(
                                    kp_pad[hp, b, h, kc, :],
                                    kp_ps[b][hp, ks],
                                    be_sb[hp, ks],
                                    mybir.AluOpType.add,
                                )
                            else:
                                nc.scalar.copy(
                                    kp_pad[hp, b, h, kc, :], kp_ps[b][hp, ks]
                                )
                    for kc in range(2):
                        for h in range(2):
                            src = vp_ps[kc][:, b * P + h * DK : b * P + (h + 1) * DK]
                            dst = vp_aug[:, kc, b, h, 0:DK]
                            if use_bf:
                                nc.vector.tensor_scalar_add(
                                    dst, src, bf_sb[:, kc : kc + 1]
                                )
                            else:
                                # DVE handles Vp so the p1->p2 staging runs
                                # on two engines concurrently
                                nc.vector.tensor_copy(dst, src)

            # ================= phase 2: scores/softmax/attn + per-b A2A ====
            with (
                tc.tile_pool(name="p2", bufs=1) as p2,
                tc.tile_pool(name="p2ps", bufs=1, space="PSUM") as p2ps,
                tc.tile_pool(name="p3", bufs=1) as p3,
                tc.tile_pool(name="p3ps", bufs=1, space="PSUM") as p3ps,
            ):
                last_at = [None] * B    # last at-matmul of each p2 batch
                last_write = [None] * B  # last a2a_in write of each batch
                last_pay = [None] * B    # last DVE payload copy of each batch
                cc_ins = [None] * B      # last collective trigger of each b
                # ======= phase 3a: per-b normalize chains ==================
                # Emitted INTERLEAVED with phase-2 batches (chain(b) right
                # after p2(b+2)'s emission) so the DVE/Pool prioritize them
                # over later payload copies — the collective stream has
                # ~13us of slack before b3's transfers, so this is free,
                # and gn(b) is ready the moment the PE finishes phase 2.
                gns_all = [None] * B

                def emit_chain(b):
                    # Pin every p3(b) instruction behind p2(b+2)'s work on
                    # the same engine/queue: the scheduler's collective-
                    # latency estimate is optimistic, and any instruction
                    # placed early that waits on a slow AllToAll head-of-
                    # line blocks its whole in-order engine stream (the
                    # p2 pipeline stalls, the next collectives trigger
                    # late, and the delay cascades).
                    ban = min(b + 2, B - 1)
                    dve_anchor = last_pay[ban]
                    pool_anchor = cc_ins[ban]
                    # The reciprocal-broadcast chain (den gather -> recip ->
                    # DRAM bounce -> broadcast) is ~12us of serialized
                    # latency gating all of p3(b); it runs on the near-idle
                    # SCALAR queue with a b+1 anchor so it completes during
                    # phase 2 instead of after it.  Worst case it only
                    # delays output stores, never the a2a pipeline.
                    chain_ban = min(b + 1, B - 1)
                    chain_anchor = last_write[chain_ban]
                    # gather all 16 denominator rows (row 64 of each source
                    # core's 65-row block; den_all row r = h*8 + src)
                    den_all = p3.tile([2 * (D // P), NSH], BF16, name="den",
                                      tag="den", bufs=2)
                    for h in range(2):
                        dg = nc.scalar.dma_start(
                            den_all[h * (D // P) : (h + 1) * (D // P), :],
                            a2a_out[b][h][:, DK, :],
                        )
                        add_dep_helper(dg.ins, chain_anchor.ins, sync=False,
                                       reason="order p3 DMAs after p2 writes")
                    rden = p3.tile([2 * (D // P), NSH], BF16, name="rden",
                                   tag="rden", bufs=2)
                    with nc.allow_low_precision(
                        reason="bf16 reciprocal of softmax denominator; "
                        "matches the bf16 a2a payload precision"
                    ):
                        rc = nc.vector.reciprocal(rden[:], den_all[:])
                    add_dep_helper(rc.ins, last_pay[chain_ban].ins, sync=False,
                                   reason="order recip after p2 payload copies")
                    rs = nc.scalar.dma_start(rden_d[b][:], rden[:])
                    add_dep_helper(rs.ins, chain_anchor.ins, sync=False,
                                   reason="order p3 DMAs after p2 writes")
                    # two stride-0 broadcast DMAs build all 16 replicated
                    # reciprocal rows: rb_all[u*64+r, dm, :] = rden[u*8+dm, :]
                    rb_all = p3.tile([P, D // P, NSH], BF16, name="rb",
                                     tag="rb", bufs=2)
                    for u in range(2):
                        bc = nc.scalar.dma_start(
                            rb_all[u * DK : (u + 1) * DK, :, :],
                            rden_d[b][u * (D // P) : (u + 1) * (D // P), :]
                            .rearrange("m (o f) -> o m f", o=1)
                            .broadcast_to([DK, D // P, NSH]),
                        )
                        add_dep_helper(bc.ins, chain_anchor.ins, sync=False,
                                       reason="order p3 DMAs after p2 writes")
                    gns = []
                    for dm in range(D // P):
                        # load the 2x64 numerator rows (den row skipped)
                        g = p3.tile([P, NSH], BF16, name="g", tag="g", bufs=16)
                        for h in range(2):
                            gld = nc.sync.dma_start(
                                g[h * DK : (h + 1) * DK, :],
                                a2a_out[b][h][dm, 0:DK, :],
                            )
                            add_dep_helper(
                                gld.ins, chain_anchor.ins, sync=False,
                                reason="order g loads after p2 writes",
                            )
                        # normalize split across Pool (h0) and DVE (h1)
                        gn = p3.tile([P, NSH], BF16, name="gn", tag="gn",
                                     bufs=16)
                        for h, eng in ((0, nc.gpsimd), (1, nc.vector)):
                            mi = eng.tensor_tensor(
                                gn[h * DK : (h + 1) * DK, :],
                                g[h * DK : (h + 1) * DK, :],
                                rb_all[h * DK : (h + 1) * DK, dm, :],
                                mybir.AluOpType.mult,
                            )
                            anchor = pool_anchor if h == 0 else dve_anchor
                            add_dep_helper(
                                mi.ins, anchor.ins, sync=False,
                                reason="order normalize mults after p2",
                            )
                        gns.append(gn)
                    gns_all[b] = gns

                for b in range(B):
                    # whole batch of Q^T resident: two 1MB loads
                    QT2 = p2.tile([P, NPH, NSH], BF16, name="QT2", tag="QT2",
                                  bufs=2)
                    for qh in range(2):
                        nc.sync.dma_start(
                            QT2[:, qh * 4 : (qh + 1) * 4, :],
                            QT_p[b, :, qh * 4 * NSH : (qh + 1) * 4 * NSH],
                        )
                    for h in range(2):
                        for nh in range(NPH):
                            if nh % 2 == 0:
                                # staged [65, nh-pair, n]; the payload DMA
                                # reorders to [dest, 65, n]
                                attn_sb = p2.tile(
                                    [DK + 1, 2, NSH], BF16, name="attn",
                                    tag="attn", bufs=4
                                )
                            # per-kc score/exp tiles: finer granularity keeps
                            # the st->exp->at chain pipelined across units
                            ET = p2.tile([P, 2, NSH], BF16, name="ET", tag="ET",
                                         bufs=3)
                            for kc in range(2):
                                st = p2ps.tile(
                                    [P, NSH], F32, name="st", tag="st", bufs=4
                                )
                                nc.tensor.matmul(
                                    st[:],
                                    kp_pad[:, b, h, kc, :],
                                    QT2[:, nh, :],
                                    start=True,
                                    stop=True,
                                )
                                nc.scalar.activation(
                                    ET[:, kc, :],
                                    st[:],
                                    mybir.ActivationFunctionType.Exp,
                                    scale=0.125,
                                )
                            # at rows 0:64 = unnormalized numerator^T,
                            # row 64 = softmax denominator (ones col),
                            # rows 65: zero padding
                            at = p2ps.tile([P, NSH], F32, name="at",
                                           tag="at", bufs=2)
                            for kc in range(2):
                                mm = nc.tensor.matmul(
                                    at[:],
                                    vp_aug[:, kc, b, h, :],
                                    ET[:, kc, :],
                                    start=(kc == 0),
                                    stop=(kc == 1),
                                )
                            last_at[b] = mm
                            # ship the whole 65-row block (num + den) bf16
                            last_pay[b] = nc.vector.tensor_copy(
                                attn_sb[:, nh % 2, :], at[0 : DK + 1, :]
                            )
                            if nh % 2 == 1:
                                last_write[b] = nc.sync.dma_start(
                                    a2a_in[b][h][nh - 1 : nh + 1, :, :]
                                    .rearrange("m v f -> v m f"),
                                    attn_sb[:],
                                )
                        cc_ins[b] = nc.gpsimd.collective_compute(
                            "AllToAll",
                            mybir.AluOpType.bypass,
                            replica_groups=rg,
                            ins=[a2a_in[b][h][:]],
                            outs=[a2a_out[b][h][:]],
                        )
                    if b == 1:
                        # Wo arrives on the scalar queue during phase 2 —
                        # emitted after batch 1 so it doesn't outprioritize
                        # the early QT2 loads; needed when p3 b0 starts.
                        for half in range(2):
                            hw = slice(half * (D // P // 2), (half + 1) * (D // P // 2))
                            nc.scalar.dma_start(Wo_sb[:, hw, :], Wo_p[:, hw, :])
                        if use_bo:
                            bo_sb = wpool.tile([P, D], F32)
                            nc.scalar.dma_start(bo_sb[:], bo_p[:])
                    if b >= 2:
                        # interleave normalize-chain emission so DVE/Pool
                        # prioritize it over later payload copies
                        emit_chain(b - 2)

                emit_chain(2)
                emit_chain(3)

                # ======= phase 3b: output projection ======================
                for b in range(B):
                    gns = gns_all[b]
                    for mt in range(NSH // P):
                        f = [
                            p3ps.tile([P, 512], F32, name=f"f{fi}", tag="f",
                                      bufs=2)
                            for fi in range(2)
                        ]
                        for dm in range(D // P):
                            for fi in range(2):
                                mm = nc.tensor.matmul(
                                    f[fi][:],
                                    gns[dm][:, mt * P : (mt + 1) * P],
                                    Wo_sb[:, dm, fi * 512 : (fi + 1) * 512],
                                    start=(dm == 0),
                                    stop=(dm == D // P - 1),
                                )
                                if dm == 0 and fi == 0 and mt == 0:
                                    # pin p3(b)'s PE work behind p2(b+2)'s:
                                    # the scheduler's CC-latency estimate is
                                    # optimistic, and an early f-matmul
                                    # waiting on a slow AllToAll head-of-line
                                    # blocks the in-order PE queue.
                                    anchor = last_at[min(b + 2, B - 1)]
                                    add_dep_helper(
                                        mm.ins,
                                        anchor.ins,
                                        sync=False,
                                        reason="order p3 PE after p2(b+2)",
                                    )
                        osb = p3.tile([P, D], F32, name="osb", tag="osb", bufs=6)
                        if use_bo:
                            for fi in range(2):
                                nc.vector.tensor_tensor(
                                    osb[:, fi * 512 : (fi + 1) * 512],
                                    f[fi][:],
                                    bo_sb[:, fi * 512 : (fi + 1) * 512],
                                    mybir.AluOpType.add,
                                )
                        else:
                            # PSUM->SBUF eviction on Act (DVE carries the
                            # phase-3 normalize multiplies)
                            nc.scalar.copy(osb[:, 0:512], f[0][:])
                            nc.scalar.copy(osb[:, 512:1024], f[1][:])
                        nc.scalar.dma_start(
                            out_p[b, mt * P : (mt + 1) * P, :], osb[:]
                        )

    return nc


def kernel(K, Q, V, We, be, Wf, bf, Wo, bo, n_heads, _trace=False):
    assert int(n_heads) == H
    K = np.asarray(K, np.float32)
    Q = np.asarray(Q, np.float32)
    V = np.asarray(V, np.float32)
    We = np.asarray(We, np.float32)
    be = np.asarray(be, np.float32)
    Wf = np.asarray(Wf, np.float32)
    bf = np.asarray(bf, np.float32)
    Wo = np.asarray(Wo, np.float32)
    bo = np.asarray(bo, np.float32)

    use_be = bool(np.any(be))
    use_bf = bool(np.any(bf))
    use_bo = bool(np.any(bo))

    key = (use_be, use_bf, use_bo)
    if key not in _BUILD_CACHE:
        _BUILD_CACHE[key] = _split_multi_waits(_build(*key))
    nc = _BUILD_CACHE[key]

    Kb = K.astype(NP_BF16)
    Vb = V.astype(NP_BF16)
    Qb = Q.astype(NP_BF16)
    WeS = np.ascontiguousarray(
        We.astype(NP_BF16).reshape(NCH, P, LK).transpose(1, 0, 2)
    )
    WfS = np.ascontiguousarray(
        Wf.astype(NP_BF16).reshape(NCH, P, LK).transpose(1, 0, 2)
    )
    WoS = np.ascontiguousarray(
        Wo.astype(NP_BF16).reshape(D // P, P, D).transpose(1, 0, 2)
    )

    in_maps = []
    for c in range(NC):
        cs = slice(P * c, P * (c + 1))
        m = {
            "Ks": np.ascontiguousarray(Kb[:, :, cs].transpose(1, 0, 2)),
            "Vs": np.ascontiguousarray(Vb[:, :, cs].transpose(1, 0, 2)),
            "QTs": np.ascontiguousarray(Qb[:, :, cs].transpose(0, 2, 1)),
            "WeS": WeS,
            "WfS": WfS,
            "WoS": WoS,
        }
        if use_be:
            m["beB"] = np.broadcast_to(be, (P, LK)).copy()
        if use_bf:
            m["bfB"] = np.ascontiguousarray(bf.reshape(2, P).T)
        if use_bo:
            m["boB"] = np.broadcast_to(bo, (P, D)).copy()
        in_maps.append(m)

    res = run_bass_kernel_spmd(nc, in_maps, list(range(NC)), trace=_trace)

    out = np.empty((B, N, D), np.float32)
    for c in range(NC):
        out[:, NSH * c : NSH * (c + 1), :] = res.results[c]["out"]
    if _trace:
        kernel._last_exec_time_ns = res.exec_time_ns
    return out


kernel._last_exec_time_ns = None
